# revision 38
# baseline (speedup 1.0000x reference)
"""Trainium2 Bass kernel for nn_CrossAttnTimeQueryHead.

Strategy: data-parallel over B (128 -> 16 per core x 8 cores), weights
replicated.  Host side does pure relayout only (shard slicing, transposes,
reshapes); all arithmetic runs on-device.

v3: fp8(e4m3) + DoubleRow core with explicit engine balancing.
  - A-form scores: A_i = Q_i @ kw_i^T folded per layer, so scores^T[t,(h,q)]
    = hT^T @ A^T comes straight from hT -- no K projection, no K copies.
    (kb dropped: softmax shift-invariant; vb folded into ob' = ob + vb@ow.)
  - hT stored fp8 as 8 zero-padded 128-col t-chunks per dc: the shared
    DoubleRow lhsT for both V-proj and scores (one LDW, two MMs per chunk).
  - attention weights (exp out, bf16) + V (bf16) feed attnV with FWL loads;
    softmax sums via a ones-column folded into the V tiles.
  - exp batched over two t-chunks ([128,1024] PSUM span); LN rstd batched
    per layer into two [128,8] ACTIVATEs so ACT table sets never ping-pong
    (exp set -> ln set -> gelu set, once per layer).
  - engines pinned: ACT = exp/gelu/rstd/posTb only; PSUM->SBUF copies split
    between DVE (tensor_copy) and Pool (scalar_tensor_tensor); x/weight
    casting DMAs on SWDGE.
  - x/win bf16 (accuracy); residual/LN/o-proj/head fp32/bf16.
"""

import sys
import os
from contextlib import ExitStack

for _p in ("/opt/trn_rl_repo",):
    if _p not in sys.path and os.path.isdir(_p):
        sys.path.insert(0, _p)

import numpy as np

import concourse.bass as bass
import concourse.mybir as mybir
import concourse.tile as tile
from concourse import bacc
from concourse import bass_utils
from concourse.masks import make_identity

F32 = mybir.dt.float32
BF16 = mybir.dt.bfloat16
FP8 = mybir.dt.float8e4
AF = mybir.ActivationFunctionType
DR = mybir.MatmulPerfMode.DoubleRow
ALU = mybir.AluOpType

N_CORES = 8
B = 128
B_LOC = B // N_CORES          # 16
T = 1000
D_IN = 512
D = 256
H = 8
HEAD = 32
L = 2
D_FF = 1024
D_OUT = 512
TQ = 64
SCALE = HEAD ** -0.5
EPS = 1e-5
KC = 8                        # t chunks (125 real + 3 pad cols each)
KCS = 125
TH = 500                      # t halves for h-proj matmuls
PAIRS = B_LOC // 2            # 8

RO_QB = 0          # qb: i*D          (2*256)
RO_F2B = 512       # f2b: + i*D
RO_BOUT = 1024     # bout (512)
RO_OB = 1536       # ob raw: + i*D
RO_OBP = 2048      # ob' = ob + vb@ow (computed on chip): + i*D
ROWS_LEN = 2560
GELU = [AF.Gelu]   # swappable for sim (CoreSim lacks Gelu)


def _emit(ctx, tc, outs, ins):
    nc = tc.nc
    out_d = outs["out"]

    def gcopy(out, src):
        # SBUF->SBUF copy (with optional cast) via SWDGE on the Pool engine
        nc.gpsimd.dma_start(out=out, in_=src)

    def scopy(out, src):
        # SBUF->SBUF same-dtype copy via HWDGE on the idle Sync engine
        nc.sync.dma_start(out=out, in_=src)

    # ---------------- pools ----------------
    consts = ctx.enter_context(tc.tile_pool(name="consts", bufs=1))
    stage_p = ctx.enter_context(tc.tile_pool(name="stage", bufs=1))
    xt_p = ctx.enter_context(tc.tile_pool(name="xt", bufs=3))
    ht_p = ctx.enter_context(tc.tile_pool(name="ht", bufs=1))
    at_p = ctx.enter_context(tc.tile_pool(name="at", bufs=4))
    vt_p = ctx.enter_context(tc.tile_pool(name="vt", bufs=4))
    a1_p = ctx.enter_context(tc.tile_pool(name="a1", bufs=1))
    q1_p = ctx.enter_context(tc.tile_pool(name="q1", bufs=2))
    ao_p = ctx.enter_context(tc.tile_pool(name="ao", bufs=3))
    aot_p = ctx.enter_context(tc.tile_pool(name="aot", bufs=2))
    qs_p = ctx.enter_context(tc.tile_pool(name="qstate", bufs=1))
    ln_p = ctx.enter_context(tc.tile_pool(name="lnout", bufs=1))
    r_p = ctx.enter_context(tc.tile_pool(name="resid", bufs=1))
    tmp_p = ctx.enter_context(tc.tile_pool(name="tmp", bufs=2))
    small_p = ctx.enter_context(tc.tile_pool(name="small", bufs=8))
    gel_p = ctx.enter_context(tc.tile_pool(name="gel", bufs=3))
    outp_p = ctx.enter_context(tc.tile_pool(name="outp", bufs=2))

    ps_big = ctx.enter_context(tc.tile_pool(name="psbig", bufs=2, space="PSUM"))
    ps_med = ctx.enter_context(tc.tile_pool(name="psmed", bufs=2, space="PSUM"))
    ps_avs = ctx.enter_context(tc.tile_pool(name="psavs", bufs=2, space="PSUM"))

    # ---------------- constants / weights ----------------
    ones_row = consts.tile([1, 512], BF16)
    nc.vector.memset(ones_row[:], 1.0)
    zero_col = consts.tile([128, 1], F32)
    nc.vector.memset(zero_col[:], 0.0)
    eps_col = consts.tile([128, 1], F32)
    nc.vector.memset(eps_col[:], EPS)
    id_sb = consts.tile([128, 128], BF16)
    make_identity(nc, id_sb[:])

    win_dr = consts.tile([128, 2, 2, 2, 128], BF16)     # [p, kp, dc, ko, m]
    nc.gpsimd.dma_start(out=win_dr[:], in_=ins["win_r"])
    vw_dr = consts.tile([128, L, 2, 256], FP8)          # [p, i, ko(dc), e]
    nc.gpsimd.dma_start(out=vw_dr[:], in_=ins["vw_r"])
    f1w_r = consts.tile([128, L, 2, 8, 128], BF16)      # [p, i, dc, fc, m]
    nc.gpsimd.dma_start(out=f1w_r[:], in_=ins["f1w_r"])
    f2w_r = consts.tile([128, L, 8, 256], BF16)         # [p, i, fc, e]
    nc.gpsimd.dma_start(out=f2w_r[:], in_=ins["f2w_r"])

    kwT_sb = consts.tile([128, L, 2, 2, 128], BF16)     # [p(e%128), i, ec, dc, m]
    nc.gpsimd.dma_start(out=kwT_sb[:], in_=ins["kwt_r"])
    qw_r = consts.tile([128, L, 2, 2, 128], BF16)       # [p, i, dc, ec, m]
    nc.gpsimd.dma_start(out=qw_r[:], in_=ins["qw_r"])
    ow_sb = consts.tile([128, L, 2, 256], BF16)         # [p(e), i, ec, d]
    nc.gpsimd.dma_start(out=ow_sb[:], in_=ins["ow_r"])
    wout_sb = consts.tile([128, 2, D_OUT], BF16)        # [p(d), dc, o]
    nc.gpsimd.dma_start(out=wout_sb[:], in_=ins["wout_r"])
    tqT_sb = consts.tile([128, 2, TQ], BF16)            # [p(d), dc, q]
    nc.gpsimd.dma_start(out=tqT_sb[:], in_=ins["tqt_r"])
    tqpair_sb = consts.tile([128, D], F32)
    nc.sync.dma_start(out=tqpair_sb[:], in_=ins["tqpair"][:, :])
    lns_sb = consts.tile([128, L, D], F32)
    lnb_sb = consts.tile([128, L, D], F32)
    for i in range(L):
        nc.sync.dma_start(out=lns_sb[:, i, :], in_=ins["lns"][i, :, :])
        nc.sync.dma_start(out=lnb_sb[:, i, :], in_=ins["lnb"][i, :, :])
    bin_col = consts.tile([128, 2], F32)
    for c in range(2):
        nc.sync.dma_start(out=bin_col[:, c:c + 1], in_=ins["binv"][c * 128:(c + 1) * 128])
    vb_col = consts.tile([128, 2 * L], BF16)
    for i in range(L):
        for ec in range(2):
            nc.gpsimd.dma_start(out=vb_col[:, i * 2 + ec: i * 2 + ec + 1],
                                in_=ins["vbv"][i, ec * 128:(ec + 1) * 128])

    rows_sb = consts.tile([1, ROWS_LEN], BF16)
    for i in range(L):
        nc.gpsimd.dma_start(out=rows_sb[0:1, RO_QB + i * D: RO_QB + (i + 1) * D],
                            in_=ins["qb"][i, :])
        nc.gpsimd.dma_start(out=rows_sb[0:1, RO_F2B + i * D: RO_F2B + (i + 1) * D],
                            in_=ins["f2b"][i, :])
        nc.gpsimd.dma_start(out=rows_sb[0:1, RO_OB + i * D: RO_OB + (i + 1) * D],
                            in_=ins["ob"][i, :])
    nc.gpsimd.dma_start(out=rows_sb[0:1, RO_BOUT: RO_BOUT + D_OUT], in_=ins["bout"][:])

    f1bT = consts.tile([4, L * 2 * 128], BF16)
    nc.gpsimd.dma_start(out=f1bT[:], in_=ins["f1bt"])
    flmask = consts.tile([4, 512], BF16)
    nc.gpsimd.dma_start(out=flmask[:], in_=ins["flmask"])
    onespad = consts.tile([128, 8], BF16)
    nc.gpsimd.dma_start(out=onespad[:], in_=ins["onespad"])

    # posTb = (pos + bin)^T  bf16 [p, dc, t]
    posTb = consts.tile([128, 2, T], BF16)
    post_st = stage_p.tile([128, 2, T], F32, tag="post")
    nc.sync.dma_start(out=post_st[:], in_=ins["post_r"])
    for dc in range(2):
        nc.scalar.activation(out=posTb[:, dc, :], in_=post_st[:, dc, :],
                             func=AF.Identity, bias=bin_col[:, dc:dc + 1], scale=1.0)

    # ob' = ob + vb @ ow  per layer -> rows_sb[RO_OBP + i*D]
    for i in range(L):
        pso = ps_med.tile([1, D], F32, tag="med")
        for ec in range(2):
            nc.tensor.matmul(pso[0:1, :], lhsT=vb_col[:, i * 2 + ec: i * 2 + ec + 1],
                             rhs=ow_sb[:, i, ec, :], start=(ec == 0), stop=False)
        nc.tensor.matmul(pso[0:1, :], lhsT=ones_row[0:1, 0:1],
                         rhs=rows_sb[0:1, RO_OB + i * D: RO_OB + (i + 1) * D],
                         start=False, stop=True)
        nc.vector.tensor_copy(out=rows_sb[0:1, RO_OBP + i * D: RO_OBP + (i + 1) * D],
                              in_=pso[0:1, :])

    # ---- layer-0 block-diag Q^T and A0^T (batch independent) ----
    qbd0 = consts.tile([128, 2, 256], BF16)
    nc.vector.memset(qbd0[:], 0.0)
    for ec in range(2):
        psq = ps_med.tile([128, TQ], F32, tag="med")
        nc.tensor.matmul(psq[:], lhsT=rows_sb[0:1, RO_QB + ec * 128: RO_QB + (ec + 1) * 128],
                         rhs=ones_row[0:1, 0:TQ], start=True, stop=False)
        for dc in range(2):
            nc.tensor.matmul(psq[:], lhsT=qw_r[:, 0, dc, ec, :],
                             rhs=tqT_sb[:, dc, :], start=False, stop=(dc == 1))
        for g in range(4):
            nc.vector.tensor_copy(out=qbd0[32 * g:32 * (g + 1), ec, g * TQ:(g + 1) * TQ],
                                  in_=psq[32 * g:32 * (g + 1), :])
    a0T = consts.tile([128, 2, 512], FP8)
    psA0 = ps_big.tile([128, 2, 512], F32, tag="big")
    for dc in range(2):
        for ec in range(2):
            nc.tensor.matmul(psA0[:, dc, ec * 256:(ec + 1) * 256],
                             lhsT=kwT_sb[:, 0, ec, dc, :],
                             rhs=qbd0[:, ec, :], start=True, stop=True)
    nc.vector.tensor_copy(out=a0T[:], in_=psA0[:])

    # ---------------- h projection (all batches) ----------------
    ht_tiles = []
    for b in range(B_LOC):
        ht = ht_p.tile([128, 2, KC, 128], FP8, tag=f"ht{b}")
        nc.vector.memset(ht[:, :, :, KCS:128], 0.0)
        ht_tiles.append(ht)

    xt_tiles = [None] * B_LOC
    for b in range(B_LOC):
        xt = xt_p.tile([128, 2, 2, 1008], BF16, tag="xt")
        nc.gpsimd.dma_start(out=xt[:, :, :, 0:T], in_=ins["xt"][b])
        xt_tiles[b] = xt

    for b in range(B_LOC):
        xt = xt_tiles[b]
        for th in range(2):
            psh = ps_big.tile([128, 2, 512], F32, tag="big")
            for dc in range(2):
                for kp in range(2):
                    for ko in range(2):
                        nc.tensor.matmul(psh[:, dc, 0:TH],
                                         lhsT=win_dr[:, kp, dc, ko, :],
                                         rhs=xt[:, kp, ko, th * TH:(th + 1) * TH],
                                         start=(kp == 0 and ko == 0),
                                         stop=(kp == 1 and ko == 1))
            for dc in range(2):
                nc.vector.tensor_add(
                    out=ht_tiles[b][:, dc, th * 4:(th + 1) * 4, 0:KCS],
                    in0=psh[:, dc, 0:TH].rearrange("p (c t) -> p c t", c=4),
                    in1=posTb[:, dc, th * TH:(th + 1) * TH].rearrange("p (c t) -> p c t", c=4))

    # ---------------- main layer loop ----------------
    qstate = [None] * PAIRS
    lnout = [None] * PAIRS
    rsb = [None] * PAIRS

    for i in range(L):
        lnmv = small_p.tile([128, 2, PAIRS], F32, tag=f"lnmv{i}", bufs=1)

        # ---- attention + o-proj + LN-stats per pair ----
        for p in range(PAIRS):
            # layer-1: per-batch A^T from q_state (interleaved per pair)
            if i == 1:
                qcast = tmp_p.tile([128, D], BF16, tag="qcast")
                gcopy(qcast[:], qstate[p][:])
                qsT = tmp_p.tile([128, 2, 128], BF16, tag="qsT")
                tpq = ps_med.tile([128, 2, 128], BF16, tag="med")
                for dc in range(2):
                    nc.tensor.transpose(tpq[:, dc, :], qcast[:, dc * 128:(dc + 1) * 128], id_sb[:])
                nc.vector.tensor_copy(out=qsT[:], in_=tpq[:])
                q1T = q1_p.tile([128, 2, 128], BF16, tag="q1T")
                for ec in range(2):
                    psq = ps_med.tile([128, 128], F32, tag="med")
                    nc.tensor.matmul(psq[:], lhsT=rows_sb[0:1, RO_QB + D + ec * 128: RO_QB + D + (ec + 1) * 128],
                                     rhs=ones_row[0:1, 0:128], start=True, stop=False)
                    for dc in range(2):
                        nc.tensor.matmul(psq[:], lhsT=qw_r[:, 1, dc, ec, :],
                                         rhs=qsT[:, dc, :], start=False, stop=(dc == 1))
                    nc.vector.tensor_copy(out=q1T[:, ec, :], in_=psq[:])
            a1T_pair = [None, None]
            if i == 1:
                for bb in range(2):
                    qbd1 = tmp_p.tile([128, 2, 256], BF16, tag="qbd1")
                    nc.vector.memset(qbd1[:], 0.0)
                    for ec in range(2):
                        for g in range(4):
                            scopy(qbd1[32 * g:32 * (g + 1), ec, g * TQ:(g + 1) * TQ],
                                  q1T[32 * g:32 * (g + 1), ec, bb * TQ:(bb + 1) * TQ])
                    psA = ps_big.tile([128, 2, 512], F32, tag="big")
                    for dc in range(2):
                        for ec in range(2):
                            nc.tensor.matmul(psA[:, dc, ec * 256:(ec + 1) * 256],
                                             lhsT=kwT_sb[:, 1, ec, dc, :],
                                             rhs=qbd1[:, ec, :], start=True, stop=True)
                    aT = a1_p.tile([128, 2, 512], FP8, tag=f"a1{2 * p + bb}")
                    nc.scalar.copy(out=aT[:], in_=psA[:])
                    a1T_pair[bb] = aT

            aoT = aot_p.tile([128, 2 * 128], BF16, tag="aoT")
            for bb in range(2):
                b = 2 * p + bb
                ht = ht_tiles[b]
                aT = a0T if i == 0 else a1T_pair[bb]
                avs = ps_avs.tile([128, 4, 65], F32, tag="avs")
                for cp in range(4):
                    pss = ps_big.tile([128, 2, 512], F32, tag="big")
                    psv = ps_med.tile([128, 2, 256], F32, tag="med")
                    for j in range(2):
                        kc = 2 * cp + j
                        lh = ht[:, :, kc, :]
                        nc.tensor.matmul(psv[:, j, :], lhsT=lh, rhs=vw_dr[:, i, :, :],
                                         start=True, stop=True, perf_mode=DR)
                        nc.tensor.matmul(pss[:, j, :], lhsT=lh, rhs=aT[:],
                                         start=True, stop=True, perf_mode=DR)
                    att = at_p.tile([128, 2, 512], BF16, tag="att")
                    nc.scalar.activation(out=att[:], in_=pss[:], func=AF.Exp,
                                         scale=SCALE, bias=zero_col[:, 0:1])
                    vt = vt_p.tile([128, 2, 4, 65], BF16, tag="vt")
                    scopy(vt[:, :, :, 64:65],
                          onespad[:].rearrange("p (j pp e) -> p j pp e", j=2, pp=4))
                    nc.vector.tensor_copy(
                        out=vt[:, :, :, 0:64],
                        in_=psv[:].rearrange("p j (pp e) -> p j pp e", pp=4))
                    for j in range(2):
                        kc = 2 * cp + j
                        for pp in range(4):
                            nc.tensor.matmul(
                                avs[:, pp, :],
                                lhsT=att[:, j, pp * 128:(pp + 1) * 128],
                                rhs=vt[:, j, pp, :],
                                start=(kc == 0 and pp == 0),
                                stop=(kc == KC - 1 and pp == 3))

                inv = small_p.tile([128, 4], F32, tag="inv")
                nc.vector.reciprocal(out=inv[:], in_=avs[:, :, 64])
                ao = ao_p.tile([64, D], BF16, tag="ao")
                for pp in range(4):
                    h1, h2 = 2 * pp, 2 * pp + 1
                    nc.vector.tensor_scalar_mul(
                        out=ao[0:64, h1 * 32:(h1 + 1) * 32],
                        in0=avs[0:64, pp, 0:32],
                        scalar1=inv[0:64, pp:pp + 1])
                    nc.vector.tensor_scalar_mul(
                        out=ao[0:64, h2 * 32:(h2 + 1) * 32],
                        in0=avs[64:128, pp, 32:64],
                        scalar1=inv[64:128, pp:pp + 1])
                tp = ps_med.tile([128, 2, TQ], BF16, tag="med")
                for c in range(2):
                    nc.tensor.transpose(tp[:, c, :], ao[0:TQ, c * 128:(c + 1) * 128],
                                        id_sb[0:TQ, 0:TQ])
                nc.vector.tensor_copy(
                    out=aoT[:].rearrange("p (c q) -> p c q", c=2)[:, :, bb * TQ:(bb + 1) * TQ],
                    in_=tp[:])

            # ---- o-proj + residual + LN stats (per pair) ----
            pso = ps_med.tile([128, 512], F32, tag="med")
            nc.tensor.matmul(pso[:, 0:D], lhsT=ones_row[0:1, 0:128],
                             rhs=rows_sb[0:1, RO_OBP + i * D: RO_OBP + (i + 1) * D],
                             start=True, stop=False)
            for ec in range(2):
                nc.tensor.matmul(pso[:, 0:D],
                                 lhsT=aoT[:, ec * 128:(ec + 1) * 128],
                                 rhs=ow_sb[:, i, ec, :],
                                 start=False, stop=(ec == 1))
            q_prev = tqpair_sb if i == 0 else qstate[p]
            r_sb = r_p.tile([128, D], F32, tag=f"r{p}")
            nc.vector.tensor_add(out=r_sb[:], in0=pso[:, 0:D], in1=q_prev[:])
            rsb[p] = r_sb
            st = small_p.tile([128, 6], F32, tag="st")
            nc.vector.bn_stats(out=st[:], in_=r_sb[:])
            nc.vector.bn_aggr(out=lnmv[:, :, p], in_=st[:])

        # ---- batched LN rstd (2 ACTIVATEs per layer) ----
        rstd_ln = small_p.tile([128, PAIRS], F32, tag=f"rln{i}", bufs=1)
        nc.scalar.activation(out=rstd_ln[:], in_=lnmv[:, 1, :], func=AF.Ln,
                             bias=eps_col[:, 0:1], scale=1.0)
        rstd_all = small_p.tile([128, PAIRS], F32, tag=f"rst{i}", bufs=1)
        nc.scalar.activation(out=rstd_all[:], in_=rstd_ln[:], func=AF.Exp,
                             bias=zero_col[:, 0:1], scale=-0.5)
        for p in range(PAIRS):
            lo = ln_p.tile([128, D], F32, tag=f"ln{p}")
            nc.vector.tensor_scalar(out=lo[:], in0=rsb[p][:],
                                    scalar1=lnmv[:, 0, p:p + 1],
                                    scalar2=rstd_all[:, p:p + 1],
                                    op0=ALU.subtract, op1=ALU.mult)
            nc.vector.tensor_mul(out=lo[:], in0=lo[:], in1=lns_sb[:, i, :])
            nc.vector.tensor_add(out=lo[:], in0=lo[:], in1=lnb_sb[:, i, :])
            lnout[p] = lo

        # ---- ffn phase (all pairs) ----
        for p in range(PAIRS):
            lo = lnout[p]
            lcast = tmp_p.tile([128, D], BF16, tag="lcast")
            gcopy(lcast[:], lo[:])
            lnT = tmp_p.tile([128, 2, 128], BF16, tag="lnT")
            tpl = ps_med.tile([128, 2, 128], BF16, tag="med")
            for dc in range(2):
                nc.tensor.transpose(tpl[:, dc, :], lcast[:, dc * 128:(dc + 1) * 128],
                                    id_sb[:])
            nc.vector.tensor_copy(out=lnT[:], in_=tpl[:])
            gel = gel_p.tile([128, 8, 128], BF16, tag="gel")
            for half in range(2):
                psf = ps_med.tile([128, 512], F32, tag="med")
                nc.tensor.matmul(psf[:], lhsT=f1bT[0:4, (i * 2 + half) * 128:(i * 2 + half + 1) * 128],
                                 rhs=flmask[0:4, :], start=True, stop=False)
                for fl in range(4):
                    fc = half * 4 + fl
                    for dc in range(2):
                        nc.tensor.matmul(
                            psf[:, fl * 128:(fl + 1) * 128],
                            lhsT=f1w_r[:, i, dc, fc, :],
                            rhs=lnT[:, dc, :],
                            start=False, stop=(fl == 3 and dc == 1))
                nc.scalar.activation(out=gel[:, half * 4:(half + 1) * 4, :],
                                     in_=psf[:], func=GELU[0], bias=zero_col[:, 0:1])
            ps2 = ps_avs.tile([128, D], F32, tag="avs")
            nc.tensor.matmul(ps2[:], lhsT=ones_row[0:1, 0:128],
                             rhs=rows_sb[0:1, RO_F2B + i * D: RO_F2B + (i + 1) * D],
                             start=True, stop=False)
            for fc in range(8):
                nc.tensor.matmul(ps2[:], lhsT=gel[:, fc, :],
                                 rhs=f2w_r[:, i, fc, :],
                                 start=False, stop=(fc == 7))
            qn = qs_p.tile([128, D], F32, tag=f"qn{p}")
            nc.vector.tensor_add(out=qn[:], in0=ps2[:], in1=lo[:])
            qstate[p] = qn

    # ---------------- head ----------------
    for p in range(PAIRS):
        qcast = tmp_p.tile([128, D], BF16, tag="hcast")
        gcopy(qcast[:], qstate[p][:])
        qfT = tmp_p.tile([128, 2, 128], BF16, tag="qfT")
        tpf = ps_med.tile([128, 2, 128], BF16, tag="med")
        for dc in range(2):
            nc.tensor.transpose(tpf[:, dc, :], qcast[:, dc * 128:(dc + 1) * 128], id_sb[:])
        nc.vector.tensor_copy(out=qfT[:], in_=tpf[:])
        psh = ps_med.tile([128, 512], F32, tag="med")
        nc.tensor.matmul(psh[:], lhsT=ones_row[0:1, 0:128],
                         rhs=rows_sb[0:1, RO_BOUT: RO_BOUT + D_OUT],
                         start=True, stop=False)
        for dc in range(2):
            nc.tensor.matmul(psh[:], lhsT=qfT[:, dc, :],
                             rhs=wout_sb[:, dc, :], start=False, stop=(dc == 1))
        osb = outp_p.tile([128, D_OUT], F32, tag="osb")
        nc.vector.tensor_copy(out=osb[:], in_=psh[:])
        nc.sync.dma_start(out=out_d[2 * p: 2 * p + 2, :, :], in_=osb[:])


_CACHE = {}


def _build():
    if "nc" in _CACHE:
        return _CACHE["nc"]
    nc = bacc.Bacc("TRN2", target_bir_lowering=False, debug=False,
                   num_devices=N_CORES)
    ins = {}

    def din(name, shape):
        ins[name] = nc.dram_tensor(name, list(shape), F32, kind="ExternalInput").ap()

    din("xt", (B_LOC, 128, 2, 2, T))
    din("win_r", (128, 2, 2, 2, 128))
    din("vw_r", (128, L, 2, 256))
    din("f1w_r", (128, L, 2, 8, 128))
    din("f2w_r", (128, L, 8, 256))
    din("kwt_r", (128, L, 2, 2, 128))
    din("qw_r", (128, L, 2, 2, 128))
    din("ow_r", (128, L, 2, 256))
    din("wout_r", (128, 2, D_OUT))
    din("tqt_r", (128, 2, TQ))
    din("tqpair", (128, D))
    din("post_r", (128, 2, T))
    din("lns", (L, 128, D))
    din("lnb", (L, 128, D))
    din("binv", (D,))
    din("vbv", (L, D))
    din("qb", (L, D))
    din("ob", (L, D))
    din("f2b", (L, D))
    din("f1bt", (4, L * 2 * 128))
    din("flmask", (4, 512))
    din("onespad", (128, 8))
    din("bout", (D_OUT,))
    outs = {"out": nc.dram_tensor("out", [B_LOC, TQ, D_OUT], F32,
                                  kind="ExternalOutput").ap()}
    with tile.TileContext(nc) as tc, ExitStack() as ctx:
        _emit(ctx, tc, outs, ins)
    nc.compile()
    _CACHE["nc"] = nc
    return nc


def make_in_maps(inputs):
    """Host-side shard/relayout (pure data movement, no arithmetic)."""
    f = lambda a: np.ascontiguousarray(np.asarray(a), dtype=np.float32)
    x = f(inputs["x"])
    tq = f(inputs["time_queries"])
    pos = f(inputs["pos_encoding"])[:T]
    win = f(inputs["win"])
    qw = f(inputs["qw"])
    kw = f(inputs["kw"])
    vw = f(inputs["vw"])
    ow = f(inputs["ow"])
    f1w, f2w = f(inputs["f1w"]), f(inputs["f2w"])
    f1b = f(inputs["f1b"])

    xt = x.transpose(0, 2, 1).reshape(B, 2, 2, 128, T).transpose(0, 3, 1, 2, 4)
    win_r = win.reshape(2, 2, 128, 2, 128).transpose(2, 0, 3, 1, 4)
    vw_r = vw.reshape(L, 2, 128, 256).transpose(2, 0, 1, 3)
    f1w_r = f1w.reshape(L, 2, 128, 8, 128).transpose(2, 0, 1, 3, 4)
    f2w_r = f2w.reshape(L, 8, 128, 256).transpose(2, 0, 1, 3)
    kwt = kw.transpose(0, 2, 1)   # [L, e, d]
    kwt_r = kwt.reshape(L, 2, 128, 2, 128).transpose(2, 0, 1, 3, 4)
    qw_r = qw.reshape(L, 2, 128, 2, 128).transpose(2, 0, 1, 3, 4)
    ow_r = ow.reshape(L, 2, 128, 256).transpose(2, 0, 1, 3)
    wout_r = f(inputs["wout"]).reshape(2, 128, D_OUT).transpose(1, 0, 2)
    tqt_r = tq.T.reshape(2, 128, TQ).transpose(1, 0, 2)
    post_r = pos.T.reshape(2, 128, T).transpose(1, 0, 2)
    f1bt = f1b.reshape(L, 2, 4, 128).transpose(2, 0, 1, 3).reshape(4, L * 2 * 128)

    base = {
        "win_r": np.ascontiguousarray(win_r),
        "vw_r": np.ascontiguousarray(vw_r),
        "f1w_r": np.ascontiguousarray(f1w_r),
        "f2w_r": np.ascontiguousarray(f2w_r),
        "kwt_r": np.ascontiguousarray(kwt_r),
        "qw_r": np.ascontiguousarray(qw_r),
        "ow_r": np.ascontiguousarray(ow_r),
        "wout_r": np.ascontiguousarray(wout_r),
        "tqt_r": np.ascontiguousarray(tqt_r),
        "tqpair": np.ascontiguousarray(np.concatenate([tq, tq], axis=0)),
        "post_r": np.ascontiguousarray(post_r),
        "lns": np.ascontiguousarray(np.broadcast_to(f(inputs["ln_s"])[:, None, :], (L, 128, D))),
        "lnb": np.ascontiguousarray(np.broadcast_to(f(inputs["ln_b"])[:, None, :], (L, 128, D))),
        "binv": f(inputs["bin_"]),
        "vbv": f(inputs["vb"]),
        "qb": f(inputs["qb"]),
        "ob": f(inputs["ob"]),
        "f2b": f(inputs["f2b"]),
        "f1bt": np.ascontiguousarray(f1bt),
        "flmask": np.kron(np.eye(4, dtype=np.float32), np.ones((1, 128), np.float32)),
        "onespad": np.concatenate([np.ones((KCS, 8), np.float32),
                                   np.zeros((128 - KCS, 8), np.float32)]),
        "bout": f(inputs["bout"]),
    }
    in_maps = []
    for c in range(N_CORES):
        m = dict(base)
        m["xt"] = np.ascontiguousarray(xt[c * B_LOC:(c + 1) * B_LOC])
        in_maps.append(m)
    return in_maps


def kernel(**inputs):
    nc = _build()
    in_maps = make_in_maps(inputs)
    res = bass_utils.run_bass_kernel_spmd(nc, in_maps, core_ids=list(range(N_CORES)))
    out = np.concatenate([r["out"] for r in res.results], axis=0)
    return out.astype(np.float32)


# revision 41
# speedup vs baseline: 1.2483x; 1.2483x over previous
"""Trainium2 Bass kernel for nn_CrossAttnTimeQueryHead.

Strategy: data-parallel over B (128 -> 16 per core x 8 cores), weights
replicated.  Host side does pure relayout only (shard slicing, transposes,
reshapes); all arithmetic runs on-device.

v3: fp8(e4m3) + DoubleRow core with explicit engine balancing.
  - A-form scores: A_i = Q_i @ kw_i^T folded per layer, so scores^T[t,(h,q)]
    = hT^T @ A^T comes straight from hT -- no K projection, no K copies.
    (kb dropped: softmax shift-invariant; vb folded into ob' = ob + vb@ow.)
  - hT stored fp8 as 8 zero-padded 128-col t-chunks per dc: the shared
    DoubleRow lhsT for both V-proj and scores (one LDW, two MMs per chunk).
  - attention weights (exp out, bf16) + V (bf16) feed attnV with FWL loads;
    softmax sums via a ones-column folded into the V tiles.
  - exp batched over two t-chunks ([128,1024] PSUM span); LN rstd batched
    per layer into two [128,8] ACTIVATEs so ACT table sets never ping-pong
    (exp set -> ln set -> gelu set, once per layer).
  - engines pinned: ACT = exp/gelu/rstd/posTb only; PSUM->SBUF copies split
    between DVE (tensor_copy) and Pool (scalar_tensor_tensor); x/weight
    casting DMAs on SWDGE.
  - x/win bf16 (accuracy); residual/LN/o-proj/head fp32/bf16.
"""

import sys
import os
from contextlib import ExitStack

for _p in ("/opt/trn_rl_repo",):
    if _p not in sys.path and os.path.isdir(_p):
        sys.path.insert(0, _p)

import numpy as np

import concourse.bass as bass
import concourse.mybir as mybir
import concourse.tile as tile
from concourse import bacc
from concourse import bass_utils
from concourse.masks import make_identity

F32 = mybir.dt.float32
BF16 = mybir.dt.bfloat16
FP8 = mybir.dt.float8e4
AF = mybir.ActivationFunctionType
DR = mybir.MatmulPerfMode.DoubleRow
ALU = mybir.AluOpType

N_CORES = 8
B = 128
B_LOC = B // N_CORES          # 16
T = 1000
D_IN = 512
D = 256
H = 8
HEAD = 32
L = 2
D_FF = 1024
D_OUT = 512
TQ = 64
SCALE = HEAD ** -0.5
EPS = 1e-5
KC = 8                        # t chunks (125 real + 3 pad cols each)
KCS = 125
TH = 500                      # t halves for h-proj matmuls
PAIRS = B_LOC // 2            # 8

RO_QB = 0          # qb: i*D          (2*256)
RO_F2B = 512       # f2b: + i*D
RO_BOUT = 1024     # bout (512)
RO_OB = 1536       # ob raw: + i*D
RO_OBP = 2048      # ob' = ob + vb@ow (computed on chip): + i*D
ROWS_LEN = 2560
GELU = [AF.Gelu]   # swappable for sim (CoreSim lacks Gelu)


def _emit(ctx, tc, outs, ins):
    nc = tc.nc
    out_d = outs["out"]

    def gcopy(out, src):
        # SBUF->SBUF copy (with optional cast) via SWDGE on the Pool engine
        nc.gpsimd.dma_start(out=out, in_=src)

    def scopy(out, src):
        # SBUF->SBUF same-dtype copy via HWDGE on the idle Sync engine
        nc.sync.dma_start(out=out, in_=src)

    # ---------------- pools ----------------
    consts = ctx.enter_context(tc.tile_pool(name="consts", bufs=1))
    stage_p = ctx.enter_context(tc.tile_pool(name="stage", bufs=1))
    xt_p = ctx.enter_context(tc.tile_pool(name="xt", bufs=3))
    ht_p = ctx.enter_context(tc.tile_pool(name="ht", bufs=1))
    at_p = ctx.enter_context(tc.tile_pool(name="at", bufs=4))
    vt_p = ctx.enter_context(tc.tile_pool(name="vt", bufs=4))
    a1_p = ctx.enter_context(tc.tile_pool(name="a1", bufs=1))
    q1_p = ctx.enter_context(tc.tile_pool(name="q1", bufs=2))
    ao_p = ctx.enter_context(tc.tile_pool(name="ao", bufs=3))
    aot_p = ctx.enter_context(tc.tile_pool(name="aot", bufs=2))
    qs_p = ctx.enter_context(tc.tile_pool(name="qstate", bufs=1))
    ln_p = ctx.enter_context(tc.tile_pool(name="lnout", bufs=1))
    r_p = ctx.enter_context(tc.tile_pool(name="resid", bufs=1))
    tmp_p = ctx.enter_context(tc.tile_pool(name="tmp", bufs=2))
    small_p = ctx.enter_context(tc.tile_pool(name="small", bufs=8))
    gel_p = ctx.enter_context(tc.tile_pool(name="gel", bufs=3))
    outp_p = ctx.enter_context(tc.tile_pool(name="outp", bufs=2))

    ps_big = ctx.enter_context(tc.tile_pool(name="psbig", bufs=2, space="PSUM"))
    ps_med = ctx.enter_context(tc.tile_pool(name="psmed", bufs=2, space="PSUM"))
    ps_avs = ctx.enter_context(tc.tile_pool(name="psavs", bufs=2, space="PSUM"))

    # ---------------- constants / weights ----------------
    ones_row = consts.tile([1, 512], BF16)
    nc.vector.memset(ones_row[:], 1.0)
    zero_col = consts.tile([128, 1], F32)
    nc.vector.memset(zero_col[:], 0.0)
    eps_col = consts.tile([128, 1], F32)
    nc.vector.memset(eps_col[:], EPS)
    id_sb = consts.tile([128, 128], BF16)
    make_identity(nc, id_sb[:])

    win_dr = consts.tile([128, 2, 2, 2, 128], BF16)     # [p, kp, dc, ko, m]
    nc.gpsimd.dma_start(out=win_dr[:], in_=ins["win_r"])
    vw_dr = consts.tile([128, L, 2, 256], FP8)          # [p, i, ko(dc), e]
    nc.gpsimd.dma_start(out=vw_dr[:], in_=ins["vw_r"])
    f1w_r = consts.tile([128, L, 2, 8, 128], BF16)      # [p, i, dc, fc, m]
    nc.gpsimd.dma_start(out=f1w_r[:], in_=ins["f1w_r"])
    f2w_r = consts.tile([128, L, 8, 256], BF16)         # [p, i, fc, e]
    nc.gpsimd.dma_start(out=f2w_r[:], in_=ins["f2w_r"])

    kwT_sb = consts.tile([128, L, 2, 2, 128], BF16)     # [p(e%128), i, ec, dc, m]
    nc.gpsimd.dma_start(out=kwT_sb[:], in_=ins["kwt_r"])
    qw_r = consts.tile([128, L, 2, 2, 128], BF16)       # [p, i, dc, ec, m]
    nc.gpsimd.dma_start(out=qw_r[:], in_=ins["qw_r"])
    ow_sb = consts.tile([128, L, 2, 256], BF16)         # [p(e), i, ec, d]
    nc.gpsimd.dma_start(out=ow_sb[:], in_=ins["ow_r"])
    wout_sb = consts.tile([128, 2, D_OUT], BF16)        # [p(d), dc, o]
    nc.gpsimd.dma_start(out=wout_sb[:], in_=ins["wout_r"])
    tqT_sb = consts.tile([128, 2, TQ], BF16)            # [p(d), dc, q]
    nc.gpsimd.dma_start(out=tqT_sb[:], in_=ins["tqt_r"])
    tqpair_sb = consts.tile([128, D], F32)
    nc.sync.dma_start(out=tqpair_sb[:], in_=ins["tqpair"][:, :])
    lns_sb = consts.tile([128, L, D], F32)
    lnb_sb = consts.tile([128, L, D], F32)
    for i in range(L):
        nc.sync.dma_start(out=lns_sb[:, i, :], in_=ins["lns"][i, :, :])
        nc.sync.dma_start(out=lnb_sb[:, i, :], in_=ins["lnb"][i, :, :])
    bin_col = consts.tile([128, 2], F32)
    for c in range(2):
        nc.sync.dma_start(out=bin_col[:, c:c + 1], in_=ins["binv"][c * 128:(c + 1) * 128])
    vb_col = consts.tile([128, 2 * L], BF16)
    for i in range(L):
        for ec in range(2):
            nc.gpsimd.dma_start(out=vb_col[:, i * 2 + ec: i * 2 + ec + 1],
                                in_=ins["vbv"][i, ec * 128:(ec + 1) * 128])

    rows_sb = consts.tile([1, ROWS_LEN], BF16)
    for i in range(L):
        nc.gpsimd.dma_start(out=rows_sb[0:1, RO_QB + i * D: RO_QB + (i + 1) * D],
                            in_=ins["qb"][i, :])
        nc.gpsimd.dma_start(out=rows_sb[0:1, RO_F2B + i * D: RO_F2B + (i + 1) * D],
                            in_=ins["f2b"][i, :])
        nc.gpsimd.dma_start(out=rows_sb[0:1, RO_OB + i * D: RO_OB + (i + 1) * D],
                            in_=ins["ob"][i, :])
    nc.gpsimd.dma_start(out=rows_sb[0:1, RO_BOUT: RO_BOUT + D_OUT], in_=ins["bout"][:])

    f1bT = consts.tile([4, L * 2 * 128], BF16)
    nc.gpsimd.dma_start(out=f1bT[:], in_=ins["f1bt"])
    flmask = consts.tile([4, 512], BF16)
    nc.gpsimd.dma_start(out=flmask[:], in_=ins["flmask"])
    onespad = consts.tile([128, 8], BF16)
    nc.gpsimd.dma_start(out=onespad[:], in_=ins["onespad"])

    # posTb = (pos + bin)^T  bf16 [p, dc, t]
    posTb = consts.tile([128, 2, T], BF16)
    post_st = stage_p.tile([128, 2, T], F32, tag="post")
    nc.sync.dma_start(out=post_st[:], in_=ins["post_r"])
    for dc in range(2):
        nc.scalar.activation(out=posTb[:, dc, :], in_=post_st[:, dc, :],
                             func=AF.Identity, bias=bin_col[:, dc:dc + 1], scale=1.0)

    # ob' = ob + vb @ ow  per layer -> rows_sb[RO_OBP + i*D]
    for i in range(L):
        pso = ps_med.tile([1, D], F32, tag="med")
        for ec in range(2):
            nc.tensor.matmul(pso[0:1, :], lhsT=vb_col[:, i * 2 + ec: i * 2 + ec + 1],
                             rhs=ow_sb[:, i, ec, :], start=(ec == 0), stop=False)
        nc.tensor.matmul(pso[0:1, :], lhsT=ones_row[0:1, 0:1],
                         rhs=rows_sb[0:1, RO_OB + i * D: RO_OB + (i + 1) * D],
                         start=False, stop=True)
        nc.vector.tensor_copy(out=rows_sb[0:1, RO_OBP + i * D: RO_OBP + (i + 1) * D],
                              in_=pso[0:1, :])

    # ---- layer-0 block-diag Q^T and A0^T (batch independent) ----
    qbd0 = consts.tile([128, 2, 256], BF16)
    nc.vector.memset(qbd0[:], 0.0)
    for ec in range(2):
        psq = ps_med.tile([128, TQ], F32, tag="med")
        nc.tensor.matmul(psq[:], lhsT=rows_sb[0:1, RO_QB + ec * 128: RO_QB + (ec + 1) * 128],
                         rhs=ones_row[0:1, 0:TQ], start=True, stop=False)
        for dc in range(2):
            nc.tensor.matmul(psq[:], lhsT=qw_r[:, 0, dc, ec, :],
                             rhs=tqT_sb[:, dc, :], start=False, stop=(dc == 1))
        for g in range(4):
            nc.vector.tensor_copy(out=qbd0[32 * g:32 * (g + 1), ec, g * TQ:(g + 1) * TQ],
                                  in_=psq[32 * g:32 * (g + 1), :])
    a0T = consts.tile([128, 2, 512], FP8)
    psA0 = ps_big.tile([128, 2, 512], F32, tag="big")
    for dc in range(2):
        for ec in range(2):
            nc.tensor.matmul(psA0[:, dc, ec * 256:(ec + 1) * 256],
                             lhsT=kwT_sb[:, 0, ec, dc, :],
                             rhs=qbd0[:, ec, :], start=True, stop=True)
    nc.vector.tensor_copy(out=a0T[:], in_=psA0[:])

    # ---------------- h projection (all batches) ----------------
    ht_tiles = []
    for b in range(B_LOC):
        ht = ht_p.tile([128, 2, KC, 128], FP8, tag=f"ht{b}")
        nc.vector.memset(ht[:, :, :, KCS:128], 0.0)
        ht_tiles.append(ht)

    xt_tiles = [None] * B_LOC
    for b in range(B_LOC):
        xt = xt_p.tile([128, 2, 2, 1008], BF16, tag="xt")
        nc.gpsimd.dma_start(out=xt[:, :, :, 0:T], in_=ins["xt"][b])
        xt_tiles[b] = xt

    for b in range(B_LOC):
        xt = xt_tiles[b]
        for th in range(2):
            psh = ps_big.tile([128, 2, 512], F32, tag="big")
            for dc in range(2):
                for kp in range(2):
                    for ko in range(2):
                        nc.tensor.matmul(psh[:, dc, 0:TH],
                                         lhsT=win_dr[:, kp, dc, ko, :],
                                         rhs=xt[:, kp, ko, th * TH:(th + 1) * TH],
                                         start=(kp == 0 and ko == 0),
                                         stop=(kp == 1 and ko == 1))
            for dc in range(2):
                nc.vector.tensor_add(
                    out=ht_tiles[b][:, dc, th * 4:(th + 1) * 4, 0:KCS],
                    in0=psh[:, dc, 0:TH].rearrange("p (c t) -> p c t", c=4),
                    in1=posTb[:, dc, th * TH:(th + 1) * TH].rearrange("p (c t) -> p c t", c=4))

    # ---------------- main layer loop ----------------
    qstate = [None] * PAIRS
    lnout = [None] * PAIRS
    rsb = [None] * PAIRS

    for i in range(L):
        lnmv = small_p.tile([128, 2, PAIRS], F32, tag=f"lnmv{i}", bufs=1)

        # Software pipeline: deferred work pops between projection steps so
        # the in-order PE queue never sits behind the ACT exp / DVE copies.
        pending = []
        stepno = [0]

        def enq(fn, delay):
            pending.append([stepno[0] + delay, fn])

        def tick():
            stepno[0] += 1
            rest = []
            for item in pending:
                if item[0] <= stepno[0]:
                    item[1]()
                else:
                    rest.append(item)
            pending[:] = rest

        def flush():
            for item in pending:
                item[1]()
            pending.clear()

        a1T_all = [None] * B_LOC
        q1T_box = [None]

        def a1_q(p):
            def fn():
                qcast = tmp_p.tile([128, D], BF16, tag="qcast")
                gcopy(qcast[:], qstate[p][:])
                qsT = tmp_p.tile([128, 2, 128], BF16, tag="qsT")
                tpq = ps_med.tile([128, 2, 128], BF16, tag="med")
                for dc in range(2):
                    nc.tensor.transpose(tpq[:, dc, :], qcast[:, dc * 128:(dc + 1) * 128], id_sb[:])
                nc.vector.tensor_copy(out=qsT[:], in_=tpq[:])
                q1T = q1_p.tile([128, 2, 128], BF16, tag="q1T")
                for ec in range(2):
                    psq = ps_med.tile([128, 128], F32, tag="med")
                    nc.tensor.matmul(psq[:], lhsT=rows_sb[0:1, RO_QB + D + ec * 128: RO_QB + D + (ec + 1) * 128],
                                     rhs=ones_row[0:1, 0:128], start=True, stop=False)
                    for dc in range(2):
                        nc.tensor.matmul(psq[:], lhsT=qw_r[:, 1, dc, ec, :],
                                         rhs=qsT[:, dc, :], start=False, stop=(dc == 1))
                    nc.vector.tensor_copy(out=q1T[:, ec, :], in_=psq[:])
                q1T_box[0] = q1T
            return fn

        def a1_b(p, bb):
            def fn():
                q1T = q1T_box[0]
                b = 2 * p + bb
                qbd1 = tmp_p.tile([128, 2, 256], BF16, tag="qbd1")
                nc.vector.memset(qbd1[:], 0.0)
                for ec in range(2):
                    for g in range(4):
                        nc.vector.tensor_copy(
                            out=qbd1[32 * g:32 * (g + 1), ec, g * TQ:(g + 1) * TQ],
                            in_=q1T[32 * g:32 * (g + 1), ec, bb * TQ:(bb + 1) * TQ])
                aT = a1_p.tile([128, 2, 512], FP8, tag=f"a1{b}")
                for dc in range(2):
                    psA = ps_med.tile([128, 512], F32, tag="med")
                    for ec in range(2):
                        nc.tensor.matmul(psA[:, ec * 256:(ec + 1) * 256],
                                         lhsT=kwT_sb[:, 1, ec, dc, :],
                                         rhs=qbd1[:, ec, :], start=True, stop=True)
                    nc.scalar.copy(out=aT[:, dc, :], in_=psA[:])
                a1T_all[b] = aT
            return fn

        if i == 1:
            # pair 0 built up front; pair p+1 builds interleave into pair p
            a1_q(0)()
            a1_b(0, 0)()
            a1_b(0, 1)()

        avs_box = {}

        def attnv(b, cp, att, vt, avs):
            def fn():
                for j in range(2):
                    kc = 2 * cp + j
                    for pp in range(4):
                        nc.tensor.matmul(
                            avs[:, pp, :],
                            lhsT=att[:, j, pp * 128:(pp + 1) * 128],
                            rhs=vt[:, j, pp, :],
                            start=(kc == 0 and pp == 0),
                            stop=(kc == KC - 1 and pp == 3))
            return fn

        def finish_batch(b, bb, avs, aoT):
            def fn():
                inv = small_p.tile([128, 4], F32, tag="inv")
                nc.vector.reciprocal(out=inv[:], in_=avs[:, :, 64])
                ao = ao_p.tile([64, D], BF16, tag="ao")
                for pp in range(4):
                    h1, h2 = 2 * pp, 2 * pp + 1
                    nc.vector.tensor_scalar_mul(
                        out=ao[0:64, h1 * 32:(h1 + 1) * 32],
                        in0=avs[0:64, pp, 0:32],
                        scalar1=inv[0:64, pp:pp + 1])
                    nc.vector.tensor_scalar_mul(
                        out=ao[0:64, h2 * 32:(h2 + 1) * 32],
                        in0=avs[64:128, pp, 32:64],
                        scalar1=inv[64:128, pp:pp + 1])
                tp = ps_med.tile([128, 2, TQ], BF16, tag="med")
                for c in range(2):
                    nc.tensor.transpose(tp[:, c, :], ao[0:TQ, c * 128:(c + 1) * 128],
                                        id_sb[0:TQ, 0:TQ])
                nc.vector.tensor_copy(
                    out=aoT[:].rearrange("p (c q) -> p c q", c=2)[:, :, bb * TQ:(bb + 1) * TQ],
                    in_=tp[:])
            return fn

        def oproj_pair(p, aoT):
            def fn():
                pso = ps_med.tile([128, 512], F32, tag="med")
                nc.tensor.matmul(pso[:, 0:D], lhsT=ones_row[0:1, 0:128],
                                 rhs=rows_sb[0:1, RO_OBP + i * D: RO_OBP + (i + 1) * D],
                                 start=True, stop=False)
                for ec in range(2):
                    nc.tensor.matmul(pso[:, 0:D],
                                     lhsT=aoT[:, ec * 128:(ec + 1) * 128],
                                     rhs=ow_sb[:, i, ec, :],
                                     start=False, stop=(ec == 1))
                q_prev = tqpair_sb if i == 0 else qstate[p]
                r_sb = r_p.tile([128, D], F32, tag=f"r{p}")
                nc.vector.tensor_add(out=r_sb[:], in0=pso[:, 0:D], in1=q_prev[:])
                rsb[p] = r_sb
                st = small_p.tile([128, 6], F32, tag="st")
                nc.vector.bn_stats(out=st[:], in_=r_sb[:])
                nc.vector.bn_aggr(out=lnmv[:, :, p], in_=st[:])
            return fn

        # ---- attention over all pairs, pipelined ----
        for p in range(PAIRS):
            aoT = aot_p.tile([128, 2 * 128], BF16, tag="aoT")
            for bb in range(2):
                b = 2 * p + bb
                ht = ht_tiles[b]
                aT = a0T if i == 0 else a1T_all[b]
                avs = ps_avs.tile([128, 4, 65], F32, tag="avs")
                for cp in range(4):
                    pss = ps_big.tile([128, 2, 512], F32, tag="big")
                    psv = ps_med.tile([128, 2, 256], F32, tag="med")
                    for j in range(2):
                        lh = ht[:, :, 2 * cp + j, :]
                        nc.tensor.matmul(psv[:, j, :], lhsT=lh, rhs=vw_dr[:, i, :, :],
                                         start=True, stop=True, perf_mode=DR)
                        nc.tensor.matmul(pss[:, j, :], lhsT=lh, rhs=aT[:],
                                         start=True, stop=True, perf_mode=DR)
                    att = at_p.tile([128, 2, 512], BF16, tag="att")
                    nc.scalar.activation(out=att[:], in_=pss[:], func=AF.Exp,
                                         scale=SCALE, bias=zero_col[:, 0:1])
                    vt = vt_p.tile([128, 2, 4, 65], BF16, tag="vt")
                    gcopy(vt[:, :, :, 64:65],
                          onespad[:].rearrange("p (j pp e) -> p j pp e", j=2, pp=4))
                    nc.vector.tensor_copy(
                        out=vt[:, :, :, 0:64],
                        in_=psv[:].rearrange("p j (pp e) -> p j pp e", pp=4))
                    enq(attnv(b, cp, att, vt, avs), 2)
                    if i == 1 and bb == 0 and p + 1 < PAIRS:
                        if cp == 0:
                            enq(a1_q(p + 1), 3)
                        elif cp == 1:
                            enq(a1_b(p + 1, 0), 3)
                        elif cp == 2:
                            enq(a1_b(p + 1, 1), 3)
                    tick()
                enq(finish_batch(b, bb, avs, aoT), 3)
            enq(oproj_pair(p, aoT), 4)
        flush()

        # ---- batched LN rstd (2 ACTIVATEs per layer) ----
        rstd_ln = small_p.tile([128, PAIRS], F32, tag=f"rln{i}", bufs=1)
        nc.scalar.activation(out=rstd_ln[:], in_=lnmv[:, 1, :], func=AF.Ln,
                             bias=eps_col[:, 0:1], scale=1.0)
        rstd_all = small_p.tile([128, PAIRS], F32, tag=f"rst{i}", bufs=1)
        nc.scalar.activation(out=rstd_all[:], in_=rstd_ln[:], func=AF.Exp,
                             bias=zero_col[:, 0:1], scale=-0.5)
        for p in range(PAIRS):
            lo = ln_p.tile([128, D], F32, tag=f"ln{p}")
            nc.vector.tensor_scalar(out=lo[:], in0=rsb[p][:],
                                    scalar1=lnmv[:, 0, p:p + 1],
                                    scalar2=rstd_all[:, p:p + 1],
                                    op0=ALU.subtract, op1=ALU.mult)
            nc.vector.tensor_mul(out=lo[:], in0=lo[:], in1=lns_sb[:, i, :])
            nc.vector.tensor_add(out=lo[:], in0=lo[:], in1=lnb_sb[:, i, :])
            lnout[p] = lo

        # ---- ffn phase (all pairs; ffn2 skewed one pair behind gelu) ----
        def ffn2(p, gel):
            lo = lnout[p]
            ps2 = ps_avs.tile([128, D], F32, tag="avs")
            nc.tensor.matmul(ps2[:], lhsT=ones_row[0:1, 0:128],
                             rhs=rows_sb[0:1, RO_F2B + i * D: RO_F2B + (i + 1) * D],
                             start=True, stop=False)
            for fc in range(8):
                nc.tensor.matmul(ps2[:], lhsT=gel[:, fc, :],
                                 rhs=f2w_r[:, i, fc, :],
                                 start=False, stop=(fc == 7))
            qn = qs_p.tile([128, D], F32, tag=f"qn{p}")
            nc.vector.tensor_add(out=qn[:], in0=ps2[:], in1=lo[:])
            qstate[p] = qn

        prev_ffn = None
        for p in range(PAIRS):
            lo = lnout[p]
            lcast = tmp_p.tile([128, D], BF16, tag="lcast")
            gcopy(lcast[:], lo[:])
            lnT = tmp_p.tile([128, 2, 128], BF16, tag="lnT")
            tpl = ps_med.tile([128, 2, 128], BF16, tag="med")
            for dc in range(2):
                nc.tensor.transpose(tpl[:, dc, :], lcast[:, dc * 128:(dc + 1) * 128],
                                    id_sb[:])
            nc.vector.tensor_copy(out=lnT[:], in_=tpl[:])
            gel = gel_p.tile([128, 8, 128], BF16, tag="gel")
            for half in range(2):
                psf = ps_med.tile([128, 512], F32, tag="med")
                nc.tensor.matmul(psf[:], lhsT=f1bT[0:4, (i * 2 + half) * 128:(i * 2 + half + 1) * 128],
                                 rhs=flmask[0:4, :], start=True, stop=False)
                for fl in range(4):
                    fc = half * 4 + fl
                    for dc in range(2):
                        nc.tensor.matmul(
                            psf[:, fl * 128:(fl + 1) * 128],
                            lhsT=f1w_r[:, i, dc, fc, :],
                            rhs=lnT[:, dc, :],
                            start=False, stop=(fl == 3 and dc == 1))
                nc.scalar.activation(out=gel[:, half * 4:(half + 1) * 4, :],
                                     in_=psf[:], func=GELU[0], bias=zero_col[:, 0:1])
            if prev_ffn is not None:
                ffn2(*prev_ffn)
            prev_ffn = (p, gel)
        ffn2(*prev_ffn)

    # ---------------- head ----------------
    for p in range(PAIRS):
        qcast = tmp_p.tile([128, D], BF16, tag="hcast")
        gcopy(qcast[:], qstate[p][:])
        qfT = tmp_p.tile([128, 2, 128], BF16, tag="qfT")
        tpf = ps_med.tile([128, 2, 128], BF16, tag="med")
        for dc in range(2):
            nc.tensor.transpose(tpf[:, dc, :], qcast[:, dc * 128:(dc + 1) * 128], id_sb[:])
        nc.vector.tensor_copy(out=qfT[:], in_=tpf[:])
        psh = ps_med.tile([128, 512], F32, tag="med")
        nc.tensor.matmul(psh[:], lhsT=ones_row[0:1, 0:128],
                         rhs=rows_sb[0:1, RO_BOUT: RO_BOUT + D_OUT],
                         start=True, stop=False)
        for dc in range(2):
            nc.tensor.matmul(psh[:], lhsT=qfT[:, dc, :],
                             rhs=wout_sb[:, dc, :], start=False, stop=(dc == 1))
        osb = outp_p.tile([128, D_OUT], F32, tag="osb")
        nc.vector.tensor_copy(out=osb[:], in_=psh[:])
        nc.sync.dma_start(out=out_d[2 * p: 2 * p + 2, :, :], in_=osb[:])


_CACHE = {}


def _build():
    if "nc" in _CACHE:
        return _CACHE["nc"]
    nc = bacc.Bacc("TRN2", target_bir_lowering=False, debug=False,
                   num_devices=N_CORES)
    ins = {}

    def din(name, shape):
        ins[name] = nc.dram_tensor(name, list(shape), F32, kind="ExternalInput").ap()

    din("xt", (B_LOC, 128, 2, 2, T))
    din("win_r", (128, 2, 2, 2, 128))
    din("vw_r", (128, L, 2, 256))
    din("f1w_r", (128, L, 2, 8, 128))
    din("f2w_r", (128, L, 8, 256))
    din("kwt_r", (128, L, 2, 2, 128))
    din("qw_r", (128, L, 2, 2, 128))
    din("ow_r", (128, L, 2, 256))
    din("wout_r", (128, 2, D_OUT))
    din("tqt_r", (128, 2, TQ))
    din("tqpair", (128, D))
    din("post_r", (128, 2, T))
    din("lns", (L, 128, D))
    din("lnb", (L, 128, D))
    din("binv", (D,))
    din("vbv", (L, D))
    din("qb", (L, D))
    din("ob", (L, D))
    din("f2b", (L, D))
    din("f1bt", (4, L * 2 * 128))
    din("flmask", (4, 512))
    din("onespad", (128, 8))
    din("bout", (D_OUT,))
    outs = {"out": nc.dram_tensor("out", [B_LOC, TQ, D_OUT], F32,
                                  kind="ExternalOutput").ap()}
    with tile.TileContext(nc) as tc, ExitStack() as ctx:
        _emit(ctx, tc, outs, ins)
    nc.compile()
    _CACHE["nc"] = nc
    return nc


def make_in_maps(inputs):
    """Host-side shard/relayout (pure data movement, no arithmetic)."""
    f = lambda a: np.ascontiguousarray(np.asarray(a), dtype=np.float32)
    x = f(inputs["x"])
    tq = f(inputs["time_queries"])
    pos = f(inputs["pos_encoding"])[:T]
    win = f(inputs["win"])
    qw = f(inputs["qw"])
    kw = f(inputs["kw"])
    vw = f(inputs["vw"])
    ow = f(inputs["ow"])
    f1w, f2w = f(inputs["f1w"]), f(inputs["f2w"])
    f1b = f(inputs["f1b"])

    xt = x.transpose(0, 2, 1).reshape(B, 2, 2, 128, T).transpose(0, 3, 1, 2, 4)
    win_r = win.reshape(2, 2, 128, 2, 128).transpose(2, 0, 3, 1, 4)
    vw_r = vw.reshape(L, 2, 128, 256).transpose(2, 0, 1, 3)
    f1w_r = f1w.reshape(L, 2, 128, 8, 128).transpose(2, 0, 1, 3, 4)
    f2w_r = f2w.reshape(L, 8, 128, 256).transpose(2, 0, 1, 3)
    kwt = kw.transpose(0, 2, 1)   # [L, e, d]
    kwt_r = kwt.reshape(L, 2, 128, 2, 128).transpose(2, 0, 1, 3, 4)
    qw_r = qw.reshape(L, 2, 128, 2, 128).transpose(2, 0, 1, 3, 4)
    ow_r = ow.reshape(L, 2, 128, 256).transpose(2, 0, 1, 3)
    wout_r = f(inputs["wout"]).reshape(2, 128, D_OUT).transpose(1, 0, 2)
    tqt_r = tq.T.reshape(2, 128, TQ).transpose(1, 0, 2)
    post_r = pos.T.reshape(2, 128, T).transpose(1, 0, 2)
    f1bt = f1b.reshape(L, 2, 4, 128).transpose(2, 0, 1, 3).reshape(4, L * 2 * 128)

    base = {
        "win_r": np.ascontiguousarray(win_r),
        "vw_r": np.ascontiguousarray(vw_r),
        "f1w_r": np.ascontiguousarray(f1w_r),
        "f2w_r": np.ascontiguousarray(f2w_r),
        "kwt_r": np.ascontiguousarray(kwt_r),
        "qw_r": np.ascontiguousarray(qw_r),
        "ow_r": np.ascontiguousarray(ow_r),
        "wout_r": np.ascontiguousarray(wout_r),
        "tqt_r": np.ascontiguousarray(tqt_r),
        "tqpair": np.ascontiguousarray(np.concatenate([tq, tq], axis=0)),
        "post_r": np.ascontiguousarray(post_r),
        "lns": np.ascontiguousarray(np.broadcast_to(f(inputs["ln_s"])[:, None, :], (L, 128, D))),
        "lnb": np.ascontiguousarray(np.broadcast_to(f(inputs["ln_b"])[:, None, :], (L, 128, D))),
        "binv": f(inputs["bin_"]),
        "vbv": f(inputs["vb"]),
        "qb": f(inputs["qb"]),
        "ob": f(inputs["ob"]),
        "f2b": f(inputs["f2b"]),
        "f1bt": np.ascontiguousarray(f1bt),
        "flmask": np.kron(np.eye(4, dtype=np.float32), np.ones((1, 128), np.float32)),
        "onespad": np.concatenate([np.ones((KCS, 8), np.float32),
                                   np.zeros((128 - KCS, 8), np.float32)]),
        "bout": f(inputs["bout"]),
    }
    in_maps = []
    for c in range(N_CORES):
        m = dict(base)
        m["xt"] = np.ascontiguousarray(xt[c * B_LOC:(c + 1) * B_LOC])
        in_maps.append(m)
    return in_maps


def kernel(**inputs):
    nc = _build()
    in_maps = make_in_maps(inputs)
    res = bass_utils.run_bass_kernel_spmd(nc, in_maps, core_ids=list(range(N_CORES)))
    out = np.concatenate([r["out"] for r in res.results], axis=0)
    return out.astype(np.float32)
